# revision 8
# baseline (speedup 1.0000x reference)
"""Time-varying all-pole IIR filter on 8 TRN2 NeuronCores (Bass/Tile).

y[t] = x[t] - sum_{j=1..32} (a[c,j,t]/a[c,0,t]) * y[t-j]
x: (32, 16, 16384) f32, a: (16, 33, 16384) f32 -> y: (32, 16, 16384) f32.

Sharding: 2 channels per core (C=16 over 8 cores), full batch B=32 and full
T per core - pure data parallelism, no collectives.

Algorithm (v2, 21971ns modeled vs 33132ns v1 baseline):
  1. T axis cut into NSEG=8 chains per channel with a DELTA=128 zero-state
     warm-up block (truncation < 1e-7); 16 chains/core, 17 blocks of L=128.
  2. Per block the host packs D' = 16*(N - I) (N = in-block banded taps,
     diagonal = -16 folded in) plus the 32-col coupling block S as one dense
     fp8 [128 x 16ch x 160B] tile, streamed over a ring of 9 SBUF tiles.
     (A skew-packed 33-wide band DMA is 2.4x less traffic and works in
     CoreSim, but per-partition-offset access patterns lower incorrectly to
     real DMA descriptors - validated broken on hardware - so dense it is.)
  3. Two-step block solve, seeded WITHOUT the coupling term, so no serial
     inter-block dependency chain exists (the missing S reaches the chained
     tail only at O(N^3); numerically validated):
       ps0 = D'@y0  (PE)        z = copy(ps0) = -16*y1   (ACT, bf16, no
                                 act-table: scale folds into the final 1/256)
       ps1 = D'@z + S@z_prev + 16*I@z   (PE, 4 matmuls/chain-block total)
       yf  = x + ps1/256        (DVE stt, bf16 out)
  4. x streamed bf16, output stored bf16 (upcast on host); end-to-end rel
     error 4.4e-3 vs the exact recurrence (hardware-validated; gate 2e-2).

Schedule: ps1 lags ps0 by PIPE=3 blocks so the in-order PE queue never
waits on ACT; per-queue DMA assignment balances SP/Pool/ACT(+ATL) at
~13.5us each against PE 13.8us busy; 3 of 17 z-copies run on DVE; stores
are quad-block with a pair+singles tail to shorten the drain.
"""

import sys

sys.path.insert(0, "/opt/trn_rl_repo")

import numpy as np
import ml_dtypes

from concourse import bacc, mybir
from concourse.bass_utils import run_bass_kernel_spmd
from concourse.tile import TileContext

BF16 = ml_dtypes.bfloat16
F8 = ml_dtypes.float8_e4m3fn

B, C, T = 32, 16, 16384
P = 32
L = 128
NCORES = 8
CLOC = C // NCORES      # 2
NSEG = 8
NCHAIN = CLOC * NSEG    # 16
SEGT = T // NSEG        # 2048
DELTA = 128
NBLK = SEGT // L + 1    # 17
NKEEP = NBLK - 1        # 16
ROW = L + P             # 160
NBAND = 9               # band-tile ring (dense reload each block)
PIPE = 3                # ps1 lags ps0 by PIPE blocks (keeps PE stall-free)

_last_exec_ns = None


def build_graph():
    nc = bacc.Bacc(detect_race_conditions=False)

    bd = nc.declare_dram_parameter(
        "bd", [NBLK, L, NCHAIN, ROW], mybir.dt.float8e4, isOutput=False
    )
    xw = nc.declare_dram_parameter(
        "xw",
        [(NBLK + 1) // 2, L, 2, NCHAIN, B],
        mybir.dt.bfloat16,
        isOutput=False,
    )
    i16 = nc.declare_dram_parameter(
        "i16", [L, L], mybir.dt.bfloat16, isOutput=False
    )
    out = nc.declare_dram_parameter(
        "out",
        [NKEEP // 4, L, 4, NCHAIN, B],
        mybir.dt.bfloat16,
        isOutput=True,
    )

    with TileContext(nc) as tc:
        with (
            tc.tile_pool(name="cst", bufs=1) as cst,
            tc.tile_pool(name="sb", bufs=9) as sb,
            tc.tile_pool(name="lp", bufs=3) as lp,
            tc.tile_pool(name="ps", bufs=4, space="PSUM") as ps,
        ):
            i16t = cst.tile([L, L], mybir.dt.bfloat16, tag="i16")
            band = [
                cst.tile(
                    [L, NCHAIN, ROW],
                    mybir.dt.float8e4,
                    tag=f"bd{i}",
                    name=f"band{i}",
                )
                for i in range(NBAND)
            ]
            xwts = [None] * NBLK
            xwp = [None] * (NBLK // 2 + 1)

            def load_xwp(p_, split=False):
                t_ = lp.tile(
                    [L, 2, NCHAIN, B],
                    mybir.dt.bfloat16,
                    tag="xwt",
                    bufs=7,
                    name=f"xwp{p_}",
                )
                n_ = min(2 * p_ + 2, NBLK) - 2 * p_
                if split:
                    for h_ in range(n_):
                        nc.gpsimd.dma_start(
                            out=t_[:, h_ : h_ + 1], in_=xw[p_, :, h_ : h_ + 1]
                        )
                else:
                    xeng = {
                        2: nc.sync, 6: nc.sync, 8: nc.sync,
                        3: nc.scalar, 4: nc.gpsimd, 5: nc.gpsimd,
                        7: nc.gpsimd,
                    }[p_]
                    xeng.dma_start(out=t_[:, 0:n_], in_=xw[p_, :, 0:n_])
                xwp[p_] = t_
                xwts[2 * p_] = (t_, 0)
                if 2 * p_ + 1 < NBLK:
                    xwts[2 * p_ + 1] = (t_, 1)

            load_xwp(0, split=True)
            load_xwp(1, split=True)
            nc.scalar.dma_start(out=band[0][:, 8:16], in_=bd[0, :, 8:16])
            nc.scalar.dma_start(out=i16t[:], in_=i16[:])

            y1s = [None] * NBLK
            yfp = [None] * (NKEEP // 2)
            for s in range(NBLK + PIPE):
                if s < NBLK:
                    bt = band[s % NBAND]
                    if True:
                        # dense band+S load (zeros included; skewed APs do not
                        # lower correctly to hardware descriptors)
                        beng = {
                            9: nc.scalar, 11: nc.scalar,
                            1: nc.sync, 2: nc.sync, 4: nc.sync, 6: nc.sync,
                            8: nc.sync, 13: nc.sync, 15: nc.sync,
                            16: nc.sync,
                            3: nc.gpsimd, 5: nc.gpsimd, 7: nc.gpsimd,
                            10: nc.gpsimd, 12: nc.gpsimd, 14: nc.gpsimd,
                        }

                        if s == 0:
                            nc.sync.dma_start(
                                out=bt[:, 0:8], in_=bd[0, :, 0:8]
                            )
                        else:
                            beng[s].dma_start(out=bt[:], in_=bd[s])
                    if s % 2 == 1 and (s + 3) // 2 <= (NBLK - 1) // 2:
                        load_xwp((s + 3) // 2)
                    xwt, xh = xwts[s]

                    # ps0 = D'@y0 ; y1 = -ps0/16 (ACT, bf16)
                    ps0 = ps.tile(
                        [L, NCHAIN, B], mybir.dt.float32, tag="ps0", bufs=5
                    )
                    for ch in range(NCHAIN):
                        nc.tensor.matmul(
                            ps0[:, ch, :],
                            bt[:, ch, 0:L],
                            xwt[:, xh, ch, :],
                            start=True,
                            stop=True,
                        )
                    # z = ps0 = -16*y1 (plain copy, no act table); the /256
                    # compensation folds into the final DVE scalar
                    y1 = sb.tile(
                        [L, NCHAIN, B], mybir.dt.bfloat16, tag="y1", bufs=6
                    )
                    if s in (3, 7, 11):
                        nc.vector.tensor_copy(y1[:], ps0[:])
                    else:
                        nc.scalar.copy(y1[:], ps0[:])
                    y1s[s] = y1

                # ps1(t) = D'@y1 + S_prev@y1_prev + 16*I@y1, t = s - PIPE
                # (t=0 is the discarded warm-up block: no ps1/yf at all)
                t = s - PIPE
                if t < 1:
                    continue
                btt = band[t % NBAND]
                y1 = y1s[t]
                ps1 = ps.tile(
                    [L, NCHAIN, B], mybir.dt.float32, tag="ps1", bufs=3
                )
                for ch in range(NCHAIN):
                    nc.tensor.matmul(
                        ps1[:, ch, :],
                        btt[:, ch, 0:L],
                        y1[:, ch, :],
                        start=True,
                        stop=False,
                        skip_group_check=True,
                    )
                    nc.tensor.matmul(
                        ps1[:, ch, :],
                        i16t[:],
                        y1[:, ch, :],
                        start=False,
                        stop=False,
                        skip_group_check=True,
                    )
                    nc.tensor.matmul(
                        ps1[0:P, ch, :],
                        band[(t - 1) % NBAND][:, ch, L:ROW],
                        y1s[t - 1][:, ch, :],
                        start=False,
                        stop=True,
                        skip_group_check=True,
                    )

                # yf = x + ps1/256 (DVE, bf16 out); quad-block stores
                ki = t - 1
                pp, hh = ki // 4, ki % 4
                if hh == 0:
                    yfp[pp] = sb.tile(
                        [L, 4, NCHAIN, B],
                        mybir.dt.bfloat16,
                        tag="yf",
                        bufs=2,
                        name=f"yf{pp}",
                    )
                xt, xth = xwts[t]
                yf_eng = nc.vector
                yf_eng.scalar_tensor_tensor(
                    out=yfp[pp][:, hh],
                    in0=ps1[:],
                    scalar=0.00390625,
                    in1=xt[:, xth],
                    op0=mybir.AluOpType.mult,
                    op1=mybir.AluOpType.add,
                )
                if pp < 3 and hh == 3:
                    eng = {0: nc.gpsimd, 1: nc.sync, 2: nc.gpsimd}[pp]
                    eng.dma_start(out=out[pp], in_=yfp[pp][:])
                elif pp == 3 and hh == 1:
                    nc.sync.dma_start(
                        out=out[pp, :, 0:2], in_=yfp[pp][:, 0:2]
                    )
                elif pp == 3 and hh == 2:
                    nc.gpsimd.dma_start(
                        out=out[pp, :, 2:3], in_=yfp[pp][:, 2:3]
                    )
                elif pp == 3 and hh == 3:
                    nc.sync.dma_start(
                        out=out[pp, :, 3:4], in_=yfp[pp][:, 3:4]
                    )
    return nc


def _host_prep(x, a):
    x = np.asarray(x, np.float32)
    a = np.asarray(a, np.float32)
    a1 = a[:, 1:, :] / a[:, :1, :]           # (C, 32, T)
    PAD0 = DELTA
    TP = T + PAD0 + L + P
    az = np.zeros((C, P, TP), np.float32)
    az[:, :, PAD0 : PAD0 + T] = a1
    xpad = np.zeros((B, C, TP), np.float32)
    xpad[:, :, PAD0 : PAD0 + T] = x

    k = np.arange(L)
    j = np.arange(1, P + 1)
    seg = np.arange(NSEG)
    s_ = np.arange(NBLK)
    # chain ch = cl*NSEG + g ; t0[g, s] = g*SEGT - DELTA + s*L
    t0 = seg[:, None] * SEGT - DELTA + s_[None, :] * L    # (NSEG, NBLK)

    # band values: BV[c, g, s, k, j-1] = 16*a_j(t0+k+j) (fp8)
    tidx = (
        t0[None, :, :, None, None]
        + k[None, None, None, :, None]
        + j[None, None, None, None, :]
        + PAD0
    )                                                      # (1,NSEG,NBLK,L,P)
    cidx = np.arange(C)[:, None, None, None, None]
    BV = (az[cidx, (j - 1)[None, None, None, None, :], tidx] * 16.0).astype(F8)

    # xw: XW[c, g, s, k, b] bf16
    txi = t0[None, :, :, None] + k[None, None, None, :] + PAD0
    XWf = xpad[:, np.arange(C)[:, None, None, None], txi]  # (B,C,NSEG,NBLK,L)
    XWf = np.moveaxis(XWf, 0, -1).astype(BF16)             # (C,NSEG,NBLK,L,B)

    i16c = (16.0 * np.eye(L)).astype(BF16)

    # dense-scatter indices: band value j of partition k -> col k+j
    sidx = np.arange(NBLK)[:, None, None, None]
    kidx = np.arange(L)[None, :, None, None]
    chidx = np.arange(NCHAIN)[None, None, :, None]
    colidx = (np.arange(L)[:, None] + np.arange(P + 1)[None, :])[
        None, :, None, :
    ]

    in_maps = []
    for r in range(NCORES):
        # BD[s, k, ch, j]: ch = cl*NSEG + g, channel c = 2r + cl
        BD = np.empty((NBLK, L, NCHAIN, P + 1), F8)
        BD[..., 0] = np.float32(-16.0)
        XW = np.zeros((NBLK + 1, L, NCHAIN, B), BF16)
        for cl in range(CLOC):
            c = 2 * r + cl
            # BV[c] : (NSEG, NBLK, L, P) -> [s, k, g, j]
            BD[:, :, cl * NSEG : (cl + 1) * NSEG, 1:] = BV[c].transpose(
                1, 2, 0, 3
            )
            XW[:NBLK, :, cl * NSEG : (cl + 1) * NSEG, :] = XWf[c].transpose(
                1, 2, 0, 3
            )
        XWP = np.ascontiguousarray(
            XW.reshape((NBLK + 1) // 2, 2, L, NCHAIN, B).transpose(
                0, 2, 1, 3, 4
            )
        )
        DNS = np.zeros((NBLK, L, NCHAIN, ROW), F8)
        DNS[sidx, kidx, chidx, colidx] = BD
        in_maps.append({"bd": DNS, "xw": XWP, "i16": i16c})
    return in_maps


def _assemble(results):
    y = np.empty((B, C, T), np.float32)
    for r in range(NCORES):
        o = np.asarray(results[r]["out"], dtype=np.float32)
        # o[pp, k, hh, ch, b] -> [si=4pp+hh, k, ch, b]
        o = o.transpose(0, 2, 1, 3, 4).reshape(NKEEP, L, CLOC, NSEG, B)
        # -> (b, cl, g, si, k)
        o = o.transpose(4, 2, 3, 0, 1).reshape(B, CLOC, T)
        y[:, 2 * r : 2 * r + CLOC, :] = o
    return y


def kernel(x, a):
    global _last_exec_ns
    nc = build_graph()
    if not nc.is_finalized():
        nc.finalize()
    in_maps = _host_prep(x, a)
    res = run_bass_kernel_spmd(nc, in_maps, core_ids=list(range(NCORES)))
    _last_exec_ns = res.exec_time_ns
    return _assemble(res.results)


# revision 9
# speedup vs baseline: 1.0106x; 1.0106x over previous
"""Time-varying all-pole IIR filter on 8 TRN2 NeuronCores (Bass/Tile).

y[t] = x[t] - sum_{j=1..32} (a[c,j,t]/a[c,0,t]) * y[t-j]
x: (32, 16, 16384) f32, a: (16, 33, 16384) f32 -> y: (32, 16, 16384) f32.

Sharding: 2 channels per core (C=16 over 8 cores), full batch B=32 and full
T per core - pure data parallelism, no collectives.

Algorithm (v2, 21971ns modeled vs 33132ns v1 baseline):
  1. T axis cut into NSEG=8 chains per channel with a DELTA=128 zero-state
     warm-up block (truncation < 1e-7); 16 chains/core, 17 blocks of L=128.
  2. Per block the host packs D' = 16*(N - I) (N = in-block banded taps,
     diagonal = -16 folded in) plus the 32-col coupling block S as one dense
     fp8 [128 x 16ch x 160B] tile, streamed over a ring of 9 SBUF tiles.
     (A skew-packed 33-wide band DMA is 2.4x less traffic and works in
     CoreSim, but per-partition-offset access patterns lower incorrectly to
     real DMA descriptors - validated broken on hardware - so dense it is.)
  3. Two-step block solve, seeded WITHOUT the coupling term, so no serial
     inter-block dependency chain exists (the missing S reaches the chained
     tail only at O(N^3); numerically validated):
       ps0 = D'@y0  (PE)        z = copy(ps0) = -16*y1   (ACT, bf16, no
                                 act-table: scale folds into the final 1/256)
       ps1 = D'@z + S@z_prev + 16*I@z   (PE, 4 matmuls/chain-block total)
       yf  = x + ps1/256        (DVE stt, bf16 out)
  4. x streamed bf16, output stored bf16 (upcast on host); end-to-end rel
     error 4.4e-3 vs the exact recurrence (hardware-validated; gate 2e-2).

Schedule: ps1 lags ps0 by PIPE=3 blocks so the in-order PE queue never
waits on ACT; per-queue DMA assignment balances SP/Pool/ACT(+ATL) at
~13.5us each against PE 13.8us busy; 3 of 17 z-copies run on DVE; stores
are quad-block with a pair+singles tail to shorten the drain.
"""

import sys

sys.path.insert(0, "/opt/trn_rl_repo")

import numpy as np
import ml_dtypes

from concourse import bacc, mybir
from concourse.bass_utils import run_bass_kernel_spmd
from concourse.tile import TileContext

BF16 = ml_dtypes.bfloat16
F8 = ml_dtypes.float8_e4m3fn

B, C, T = 32, 16, 16384
P = 32
L = 128
NCORES = 8
CLOC = C // NCORES      # 2
NSEG = 8
NCHAIN = CLOC * NSEG    # 16
SEGT = T // NSEG        # 2048
DELTA = 128
NBLK = SEGT // L + 1    # 17
NKEEP = NBLK - 1        # 16
ROW = L + P             # 160
NBAND = 9               # band-tile ring (dense reload each block)
PIPE = 3                # ps1 lags ps0 by PIPE blocks (keeps PE stall-free)

_last_exec_ns = None


def build_graph():
    nc = bacc.Bacc(detect_race_conditions=False)

    bd = nc.declare_dram_parameter(
        "bd", [NBLK, L, NCHAIN, ROW], mybir.dt.float8e4, isOutput=False
    )
    xw = nc.declare_dram_parameter(
        "xw",
        [(NBLK + 1) // 2, L, 2, NCHAIN, B],
        mybir.dt.bfloat16,
        isOutput=False,
    )
    i16 = nc.declare_dram_parameter(
        "i16", [L, L], mybir.dt.bfloat16, isOutput=False
    )
    out = nc.declare_dram_parameter(
        "out",
        [NKEEP // 4, L, 4, NCHAIN, B],
        mybir.dt.bfloat16,
        isOutput=True,
    )

    with TileContext(nc) as tc:
        with (
            tc.tile_pool(name="cst", bufs=1) as cst,
            tc.tile_pool(name="sb", bufs=9) as sb,
            tc.tile_pool(name="lp", bufs=3) as lp,
            tc.tile_pool(name="ps", bufs=4, space="PSUM") as ps,
        ):
            i16t = cst.tile([L, L], mybir.dt.bfloat16, tag="i16")
            band = [
                cst.tile(
                    [L, NCHAIN, ROW],
                    mybir.dt.float8e4,
                    tag=f"bd{i}",
                    name=f"band{i}",
                )
                for i in range(NBAND)
            ]
            xwts = [None] * NBLK
            xwp = [None] * (NBLK // 2 + 1)

            def load_xwp(p_, split=False):
                t_ = lp.tile(
                    [L, 2, NCHAIN, B],
                    mybir.dt.bfloat16,
                    tag="xwt",
                    bufs=7,
                    name=f"xwp{p_}",
                )
                n_ = min(2 * p_ + 2, NBLK) - 2 * p_
                if split:
                    for h_ in range(n_):
                        nc.gpsimd.dma_start(
                            out=t_[:, h_ : h_ + 1], in_=xw[p_, :, h_ : h_ + 1]
                        )
                else:
                    xeng = {
                        1: nc.gpsimd,
                        2: nc.sync, 6: nc.sync, 8: nc.sync,
                        3: nc.scalar, 4: nc.gpsimd, 5: nc.gpsimd,
                        7: nc.gpsimd,
                    }[p_]
                    xeng.dma_start(out=t_[:, 0:n_], in_=xw[p_, :, 0:n_])
                xwp[p_] = t_
                xwts[2 * p_] = (t_, 0)
                if 2 * p_ + 1 < NBLK:
                    xwts[2 * p_ + 1] = (t_, 1)

            nc.gpsimd.dma_start(out=band[0][:, 8:16], in_=bd[0, :, 8:16])
            load_xwp(0, split=True)
            load_xwp(1)
            nc.scalar.dma_start(out=band[2][:], in_=bd[2])
            nc.scalar.dma_start(out=i16t[:], in_=i16[:])

            y1s = [None] * NBLK
            yfp = [None] * (NKEEP // 2)
            for s in range(NBLK + PIPE):
                if s < NBLK:
                    bt = band[s % NBAND]
                    if True:
                        # dense band+S load (zeros included; skewed APs do not
                        # lower correctly to hardware descriptors)
                        beng = {
                            9: nc.scalar, 11: nc.scalar,
                            1: nc.sync, 2: nc.sync, 3: nc.sync, 6: nc.sync,
                            8: nc.sync, 13: nc.sync, 15: nc.sync,
                            16: nc.sync,
                            4: nc.gpsimd, 5: nc.gpsimd, 7: nc.gpsimd,
                            10: nc.gpsimd, 12: nc.gpsimd, 14: nc.gpsimd,
                        }

                        if s == 0:
                            nc.sync.dma_start(
                                out=bt[:, 0:8], in_=bd[0, :, 0:8]
                            )
                        elif s != 2:
                            beng[s].dma_start(out=bt[:], in_=bd[s])
                    if s % 2 == 1 and (s + 3) // 2 <= (NBLK - 1) // 2:
                        load_xwp((s + 3) // 2)
                    xwt, xh = xwts[s]

                    # ps0 = D'@y0 ; y1 = -ps0/16 (ACT, bf16)
                    ps0 = ps.tile(
                        [L, NCHAIN, B], mybir.dt.float32, tag="ps0", bufs=5
                    )
                    for ch in range(NCHAIN):
                        nc.tensor.matmul(
                            ps0[:, ch, :],
                            bt[:, ch, 0:L],
                            xwt[:, xh, ch, :],
                            start=True,
                            stop=True,
                        )
                    # z = ps0 = -16*y1 (plain copy, no act table); the /256
                    # compensation folds into the final DVE scalar
                    y1 = sb.tile(
                        [L, NCHAIN, B], mybir.dt.bfloat16, tag="y1", bufs=6
                    )
                    if s in (3, 7, 11):
                        nc.vector.tensor_copy(y1[:], ps0[:])
                    else:
                        nc.scalar.copy(y1[:], ps0[:])
                    y1s[s] = y1

                # ps1(t) = D'@y1 + S_prev@y1_prev + 16*I@y1, t = s - PIPE
                # (t=0 is the discarded warm-up block: no ps1/yf at all)
                t = s - PIPE
                if t < 1:
                    continue
                btt = band[t % NBAND]
                y1 = y1s[t]
                ps1 = ps.tile(
                    [L, NCHAIN, B], mybir.dt.float32, tag="ps1", bufs=3
                )
                for ch in range(NCHAIN):
                    nc.tensor.matmul(
                        ps1[:, ch, :],
                        btt[:, ch, 0:L],
                        y1[:, ch, :],
                        start=True,
                        stop=False,
                        skip_group_check=True,
                    )
                    nc.tensor.matmul(
                        ps1[:, ch, :],
                        i16t[:],
                        y1[:, ch, :],
                        start=False,
                        stop=False,
                        skip_group_check=True,
                    )
                    nc.tensor.matmul(
                        ps1[0:P, ch, :],
                        band[(t - 1) % NBAND][:, ch, L:ROW],
                        y1s[t - 1][:, ch, :],
                        start=False,
                        stop=True,
                        skip_group_check=True,
                    )

                # yf = x + ps1/256 (DVE, bf16 out); quad-block stores
                ki = t - 1
                pp, hh = ki // 4, ki % 4
                if hh == 0:
                    yfp[pp] = sb.tile(
                        [L, 4, NCHAIN, B],
                        mybir.dt.bfloat16,
                        tag="yf",
                        bufs=2,
                        name=f"yf{pp}",
                    )
                xt, xth = xwts[t]
                yf_eng = nc.vector
                yf_eng.scalar_tensor_tensor(
                    out=yfp[pp][:, hh],
                    in0=ps1[:],
                    scalar=0.00390625,
                    in1=xt[:, xth],
                    op0=mybir.AluOpType.mult,
                    op1=mybir.AluOpType.add,
                )
                if pp < 3 and hh == 3:
                    eng = {0: nc.gpsimd, 1: nc.sync, 2: nc.gpsimd}[pp]
                    eng.dma_start(out=out[pp], in_=yfp[pp][:])
                elif pp == 3 and hh == 1:
                    nc.sync.dma_start(
                        out=out[pp, :, 0:2], in_=yfp[pp][:, 0:2]
                    )
                elif pp == 3 and hh == 2:
                    nc.gpsimd.dma_start(
                        out=out[pp, :, 2:3], in_=yfp[pp][:, 2:3]
                    )
                elif pp == 3 and hh == 3:
                    nc.sync.dma_start(
                        out=out[pp, :, 3:4], in_=yfp[pp][:, 3:4]
                    )
    return nc


def _host_prep(x, a):
    x = np.asarray(x, np.float32)
    a = np.asarray(a, np.float32)
    a1 = a[:, 1:, :] / a[:, :1, :]           # (C, 32, T)
    PAD0 = DELTA
    TP = T + PAD0 + L + P
    az = np.zeros((C, P, TP), np.float32)
    az[:, :, PAD0 : PAD0 + T] = a1
    xpad = np.zeros((B, C, TP), np.float32)
    xpad[:, :, PAD0 : PAD0 + T] = x

    k = np.arange(L)
    j = np.arange(1, P + 1)
    seg = np.arange(NSEG)
    s_ = np.arange(NBLK)
    # chain ch = cl*NSEG + g ; t0[g, s] = g*SEGT - DELTA + s*L
    t0 = seg[:, None] * SEGT - DELTA + s_[None, :] * L    # (NSEG, NBLK)

    # band values: BV[c, g, s, k, j-1] = 16*a_j(t0+k+j) (fp8)
    tidx = (
        t0[None, :, :, None, None]
        + k[None, None, None, :, None]
        + j[None, None, None, None, :]
        + PAD0
    )                                                      # (1,NSEG,NBLK,L,P)
    cidx = np.arange(C)[:, None, None, None, None]
    BV = (az[cidx, (j - 1)[None, None, None, None, :], tidx] * 16.0).astype(F8)

    # xw: XW[c, g, s, k, b] bf16
    txi = t0[None, :, :, None] + k[None, None, None, :] + PAD0
    XWf = xpad[:, np.arange(C)[:, None, None, None], txi]  # (B,C,NSEG,NBLK,L)
    XWf = np.moveaxis(XWf, 0, -1).astype(BF16)             # (C,NSEG,NBLK,L,B)

    i16c = (16.0 * np.eye(L)).astype(BF16)

    # dense-scatter indices: band value j of partition k -> col k+j
    sidx = np.arange(NBLK)[:, None, None, None]
    kidx = np.arange(L)[None, :, None, None]
    chidx = np.arange(NCHAIN)[None, None, :, None]
    colidx = (np.arange(L)[:, None] + np.arange(P + 1)[None, :])[
        None, :, None, :
    ]

    in_maps = []
    for r in range(NCORES):
        # BD[s, k, ch, j]: ch = cl*NSEG + g, channel c = 2r + cl
        BD = np.empty((NBLK, L, NCHAIN, P + 1), F8)
        BD[..., 0] = np.float32(-16.0)
        XW = np.zeros((NBLK + 1, L, NCHAIN, B), BF16)
        for cl in range(CLOC):
            c = 2 * r + cl
            # BV[c] : (NSEG, NBLK, L, P) -> [s, k, g, j]
            BD[:, :, cl * NSEG : (cl + 1) * NSEG, 1:] = BV[c].transpose(
                1, 2, 0, 3
            )
            XW[:NBLK, :, cl * NSEG : (cl + 1) * NSEG, :] = XWf[c].transpose(
                1, 2, 0, 3
            )
        XWP = np.ascontiguousarray(
            XW.reshape((NBLK + 1) // 2, 2, L, NCHAIN, B).transpose(
                0, 2, 1, 3, 4
            )
        )
        DNS = np.zeros((NBLK, L, NCHAIN, ROW), F8)
        DNS[sidx, kidx, chidx, colidx] = BD
        in_maps.append({"bd": DNS, "xw": XWP, "i16": i16c})
    return in_maps


def _assemble(results):
    y = np.empty((B, C, T), np.float32)
    for r in range(NCORES):
        o = np.asarray(results[r]["out"], dtype=np.float32)
        # o[pp, k, hh, ch, b] -> [si=4pp+hh, k, ch, b]
        o = o.transpose(0, 2, 1, 3, 4).reshape(NKEEP, L, CLOC, NSEG, B)
        # -> (b, cl, g, si, k)
        o = o.transpose(4, 2, 3, 0, 1).reshape(B, CLOC, T)
        y[:, 2 * r : 2 * r + CLOC, :] = o
    return y


def kernel(x, a):
    global _last_exec_ns
    nc = build_graph()
    if not nc.is_finalized():
        nc.finalize()
    in_maps = _host_prep(x, a)
    res = run_bass_kernel_spmd(nc, in_maps, core_ids=list(range(NCORES)))
    _last_exec_ns = res.exec_time_ns
    return _assemble(res.results)


# revision 10
# speedup vs baseline: 1.0339x; 1.0231x over previous
"""Time-varying all-pole IIR filter on 8 TRN2 NeuronCores (Bass/Tile).

y[t] = x[t] - sum_{j=1..32} (a[c,j,t]/a[c,0,t]) * y[t-j]
x: (32, 16, 16384) f32, a: (16, 33, 16384) f32 -> y: (32, 16, 16384) f32.

Sharding: 2 channels per core (C=16 over 8 cores), full batch B=32 and full
T per core - pure data parallelism, no collectives.

Algorithm (v2, 21250ns modeled vs 33132ns v1 baseline):
  1. T axis cut into NSEG=8 chains per channel with a DELTA=128 zero-state
     warm-up block (truncation < 1e-7); 16 chains/core, 17 blocks of L=128.
  2. Per block the host packs D' = 16*(N - I) (N = in-block banded taps,
     diagonal = -16 folded in) plus the 32-col coupling block S as one dense
     fp8 [128 x 16ch x 160B] tile, streamed over a ring of 9 SBUF tiles.
     (A skew-packed 33-wide band DMA is 2.4x less traffic and works in
     CoreSim, but per-partition-offset access patterns lower incorrectly to
     real DMA descriptors - validated broken on hardware - so dense it is.)
  3. Two-step block solve, seeded WITHOUT the coupling term, so no serial
     inter-block dependency chain exists (the missing S reaches the chained
     tail only at O(N^3); numerically validated):
       ps0 = D'@y0  (PE)        z = copy(ps0) = -16*y1   (ACT, bf16, no
                                 act-table: scale folds into the final 1/256)
       ps1 = D'@z + S@z_prev + 16*I@z   (PE, 4 matmuls/chain-block total)
       yf  = x + ps1/256        (DVE stt, bf16 out)
  4. x streamed bf16, output stored bf16 (upcast on host); end-to-end rel
     error 4.4e-3 vs the exact recurrence (hardware-validated; gate 2e-2).

Schedule: ps1 lags ps0 by PIPE=3 blocks so the in-order PE queue never
waits on ACT; per-queue DMA assignment balances SP/Pool/ACT(+ATL) at
~13.5us each against PE 13.8us busy (z-copies: ACT except 3 on DVE; they
must stay off GPSIMD, whose ops cannot read PSUM on real hardware);
stores are quad-block with a pair+singles tail; startup load order is
tuned to PE's in-order band consumption (band3 on SP ahead of band4).
"""

import sys

sys.path.insert(0, "/opt/trn_rl_repo")

import numpy as np
import ml_dtypes

from concourse import bacc, mybir
from concourse.bass_utils import run_bass_kernel_spmd
from concourse.tile import TileContext

BF16 = ml_dtypes.bfloat16
F8 = ml_dtypes.float8_e4m3fn

B, C, T = 32, 16, 16384
P = 32
L = 128
NCORES = 8
CLOC = C // NCORES      # 2
NSEG = 8
NCHAIN = CLOC * NSEG    # 16
SEGT = T // NSEG        # 2048
DELTA = 128
NBLK = SEGT // L + 1    # 17
NKEEP = NBLK - 1        # 16
ROW = L + P             # 160
NBAND = 9               # band-tile ring (dense reload each block)
PIPE = 3                # ps1 lags ps0 by PIPE blocks (keeps PE stall-free)

_last_exec_ns = None


def build_graph():
    nc = bacc.Bacc(detect_race_conditions=False)

    bd = nc.declare_dram_parameter(
        "bd", [NBLK, L, NCHAIN, ROW], mybir.dt.float8e4, isOutput=False
    )
    xw = nc.declare_dram_parameter(
        "xw",
        [(NBLK + 1) // 2, L, 2, NCHAIN, B],
        mybir.dt.bfloat16,
        isOutput=False,
    )
    i16 = nc.declare_dram_parameter(
        "i16", [L, L], mybir.dt.bfloat16, isOutput=False
    )
    out = nc.declare_dram_parameter(
        "out",
        [NKEEP // 4, L, 4, NCHAIN, B],
        mybir.dt.bfloat16,
        isOutput=True,
    )

    with TileContext(nc) as tc:
        with (
            tc.tile_pool(name="cst", bufs=1) as cst,
            tc.tile_pool(name="sb", bufs=9) as sb,
            tc.tile_pool(name="lp", bufs=3) as lp,
            tc.tile_pool(name="ps", bufs=4, space="PSUM") as ps,
        ):
            i16t = cst.tile([L, L], mybir.dt.bfloat16, tag="i16")
            band = [
                cst.tile(
                    [L, NCHAIN, ROW],
                    mybir.dt.float8e4,
                    tag=f"bd{i}",
                    name=f"band{i}",
                )
                for i in range(NBAND)
            ]
            xwts = [None] * NBLK
            xwp = [None] * (NBLK // 2 + 1)

            def load_xwp(p_, split=False):
                t_ = lp.tile(
                    [L, 2, NCHAIN, B],
                    mybir.dt.bfloat16,
                    tag="xwt",
                    bufs=7,
                    name=f"xwp{p_}",
                )
                n_ = min(2 * p_ + 2, NBLK) - 2 * p_
                if split:
                    for h_ in range(n_):
                        nc.gpsimd.dma_start(
                            out=t_[:, h_ : h_ + 1], in_=xw[p_, :, h_ : h_ + 1]
                        )
                else:
                    xeng = {
                        1: nc.gpsimd,
                        2: nc.sync, 6: nc.sync, 8: nc.sync,
                        3: nc.scalar, 4: nc.gpsimd, 5: nc.gpsimd,
                        7: nc.gpsimd,
                    }[p_]
                    xeng.dma_start(out=t_[:, 0:n_], in_=xw[p_, :, 0:n_])
                xwp[p_] = t_
                xwts[2 * p_] = (t_, 0)
                if 2 * p_ + 1 < NBLK:
                    xwts[2 * p_ + 1] = (t_, 1)

            nc.gpsimd.dma_start(out=band[0][:, 8:16], in_=bd[0, :, 8:16])
            load_xwp(0, split=True)
            load_xwp(1)
            nc.scalar.dma_start(out=band[2][:], in_=bd[2])
            nc.scalar.dma_start(out=i16t[:], in_=i16[:])

            y1s = [None] * NBLK
            yfp = [None] * (NKEEP // 2)
            for s in range(NBLK + PIPE):
                if s < NBLK:
                    bt = band[s % NBAND]
                    if True:
                        # dense band+S load (zeros included; skewed APs do not
                        # lower correctly to hardware descriptors)
                        beng = {
                            9: nc.scalar, 11: nc.scalar,
                            1: nc.sync, 2: nc.sync, 3: nc.sync, 6: nc.sync,
                            8: nc.sync, 13: nc.sync, 15: nc.sync,
                            16: nc.sync,
                            4: nc.gpsimd, 5: nc.gpsimd, 7: nc.gpsimd,
                            10: nc.gpsimd, 12: nc.gpsimd, 14: nc.gpsimd,
                        }

                        if s == 0:
                            nc.sync.dma_start(
                                out=bt[:, 0:8], in_=bd[0, :, 0:8]
                            )
                        elif s != 2:
                            beng[s].dma_start(out=bt[:], in_=bd[s])
                    if s % 2 == 1 and (s + 3) // 2 <= (NBLK - 1) // 2:
                        load_xwp((s + 3) // 2)
                    xwt, xh = xwts[s]

                    # ps0 = D'@y0 ; y1 = -ps0/16 (ACT, bf16)
                    ps0 = ps.tile(
                        [L, NCHAIN, B], mybir.dt.float32, tag="ps0", bufs=5
                    )
                    for ch in range(NCHAIN):
                        nc.tensor.matmul(
                            ps0[:, ch, :],
                            bt[:, ch, 0:L],
                            xwt[:, xh, ch, :],
                            start=True,
                            stop=True,
                        )
                    # z = ps0 = -16*y1 (plain copy, no act table); the /256
                    # compensation folds into the final DVE scalar
                    y1 = sb.tile(
                        [L, NCHAIN, B], mybir.dt.bfloat16, tag="y1", bufs=6
                    )
                    if s in (3, 7, 11):
                        nc.vector.tensor_copy(y1[:], ps0[:])
                    else:
                        nc.scalar.copy(y1[:], ps0[:])
                    y1s[s] = y1

                # ps1(t) = D'@y1 + S_prev@y1_prev + 16*I@y1, t = s - PIPE
                # (t=0 is the discarded warm-up block: no ps1/yf at all)
                t = s - PIPE
                if t < 1:
                    continue
                btt = band[t % NBAND]
                y1 = y1s[t]
                ps1 = ps.tile(
                    [L, NCHAIN, B], mybir.dt.float32, tag="ps1", bufs=3
                )
                for ch in range(NCHAIN):
                    nc.tensor.matmul(
                        ps1[:, ch, :],
                        btt[:, ch, 0:L],
                        y1[:, ch, :],
                        start=True,
                        stop=False,
                        skip_group_check=True,
                    )
                    nc.tensor.matmul(
                        ps1[:, ch, :],
                        i16t[:],
                        y1[:, ch, :],
                        start=False,
                        stop=False,
                        skip_group_check=True,
                    )
                    nc.tensor.matmul(
                        ps1[0:P, ch, :],
                        band[(t - 1) % NBAND][:, ch, L:ROW],
                        y1s[t - 1][:, ch, :],
                        start=False,
                        stop=True,
                        skip_group_check=True,
                    )

                # yf = x + ps1/256 (DVE, bf16 out); quad-block stores
                ki = t - 1
                pp, hh = ki // 4, ki % 4
                if hh == 0:
                    yfp[pp] = sb.tile(
                        [L, 4, NCHAIN, B],
                        mybir.dt.bfloat16,
                        tag="yf",
                        bufs=2,
                        name=f"yf{pp}",
                    )
                xt, xth = xwts[t]
                yf_eng = nc.vector
                yf_eng.scalar_tensor_tensor(
                    out=yfp[pp][:, hh],
                    in0=ps1[:],
                    scalar=0.00390625,
                    in1=xt[:, xth],
                    op0=mybir.AluOpType.mult,
                    op1=mybir.AluOpType.add,
                )
                if pp < 3 and hh == 3:
                    eng = {0: nc.gpsimd, 1: nc.sync, 2: nc.gpsimd}[pp]
                    eng.dma_start(out=out[pp], in_=yfp[pp][:])
                elif pp == 3 and hh == 1:
                    nc.sync.dma_start(
                        out=out[pp, :, 0:2], in_=yfp[pp][:, 0:2]
                    )
                elif pp == 3 and hh == 2:
                    nc.gpsimd.dma_start(
                        out=out[pp, :, 2:3], in_=yfp[pp][:, 2:3]
                    )
                elif pp == 3 and hh == 3:
                    nc.sync.dma_start(
                        out=out[pp, :, 3:4], in_=yfp[pp][:, 3:4]
                    )
    return nc


def _host_prep(x, a):
    x = np.asarray(x, np.float32)
    a = np.asarray(a, np.float32)
    a1 = a[:, 1:, :] / a[:, :1, :]           # (C, 32, T)
    PAD0 = DELTA
    TP = T + PAD0 + L + P
    az = np.zeros((C, P, TP), np.float32)
    az[:, :, PAD0 : PAD0 + T] = a1
    xpad = np.zeros((B, C, TP), np.float32)
    xpad[:, :, PAD0 : PAD0 + T] = x

    k = np.arange(L)
    j = np.arange(1, P + 1)
    seg = np.arange(NSEG)
    s_ = np.arange(NBLK)
    # chain ch = cl*NSEG + g ; t0[g, s] = g*SEGT - DELTA + s*L
    t0 = seg[:, None] * SEGT - DELTA + s_[None, :] * L    # (NSEG, NBLK)

    # band values: BV[c, g, s, k, j-1] = 16*a_j(t0+k+j) (fp8)
    tidx = (
        t0[None, :, :, None, None]
        + k[None, None, None, :, None]
        + j[None, None, None, None, :]
        + PAD0
    )                                                      # (1,NSEG,NBLK,L,P)
    cidx = np.arange(C)[:, None, None, None, None]
    BV = (az[cidx, (j - 1)[None, None, None, None, :], tidx] * 16.0).astype(F8)

    # xw: XW[c, g, s, k, b] bf16
    txi = t0[None, :, :, None] + k[None, None, None, :] + PAD0
    XWf = xpad[:, np.arange(C)[:, None, None, None], txi]  # (B,C,NSEG,NBLK,L)
    XWf = np.moveaxis(XWf, 0, -1).astype(BF16)             # (C,NSEG,NBLK,L,B)

    i16c = (16.0 * np.eye(L)).astype(BF16)

    # dense-scatter indices: band value j of partition k -> col k+j
    sidx = np.arange(NBLK)[:, None, None, None]
    kidx = np.arange(L)[None, :, None, None]
    chidx = np.arange(NCHAIN)[None, None, :, None]
    colidx = (np.arange(L)[:, None] + np.arange(P + 1)[None, :])[
        None, :, None, :
    ]

    in_maps = []
    for r in range(NCORES):
        # BD[s, k, ch, j]: ch = cl*NSEG + g, channel c = 2r + cl
        BD = np.empty((NBLK, L, NCHAIN, P + 1), F8)
        BD[..., 0] = np.float32(-16.0)
        XW = np.zeros((NBLK + 1, L, NCHAIN, B), BF16)
        for cl in range(CLOC):
            c = 2 * r + cl
            # BV[c] : (NSEG, NBLK, L, P) -> [s, k, g, j]
            BD[:, :, cl * NSEG : (cl + 1) * NSEG, 1:] = BV[c].transpose(
                1, 2, 0, 3
            )
            XW[:NBLK, :, cl * NSEG : (cl + 1) * NSEG, :] = XWf[c].transpose(
                1, 2, 0, 3
            )
        XWP = np.ascontiguousarray(
            XW.reshape((NBLK + 1) // 2, 2, L, NCHAIN, B).transpose(
                0, 2, 1, 3, 4
            )
        )
        DNS = np.zeros((NBLK, L, NCHAIN, ROW), F8)
        DNS[sidx, kidx, chidx, colidx] = BD
        in_maps.append({"bd": DNS, "xw": XWP, "i16": i16c})
    return in_maps


def _assemble(results):
    y = np.empty((B, C, T), np.float32)
    for r in range(NCORES):
        o = np.asarray(results[r]["out"], dtype=np.float32)
        # o[pp, k, hh, ch, b] -> [si=4pp+hh, k, ch, b]
        o = o.transpose(0, 2, 1, 3, 4).reshape(NKEEP, L, CLOC, NSEG, B)
        # -> (b, cl, g, si, k)
        o = o.transpose(4, 2, 3, 0, 1).reshape(B, CLOC, T)
        y[:, 2 * r : 2 * r + CLOC, :] = o
    return y


def kernel(x, a):
    global _last_exec_ns
    nc = build_graph()
    if not nc.is_finalized():
        nc.finalize()
    in_maps = _host_prep(x, a)
    res = run_bass_kernel_spmd(nc, in_maps, core_ids=list(range(NCORES)))
    _last_exec_ns = res.exec_time_ns
    return _assemble(res.results)


# revision 11
# speedup vs baseline: 1.0659x; 1.0310x over previous
"""Time-varying all-pole IIR filter on 8 TRN2 NeuronCores (Bass/Tile).

y[t] = x[t] - sum_{j=1..32} (a[c,j,t]/a[c,0,t]) * y[t-j]
x: (32, 16, 16384) f32, a: (16, 33, 16384) f32 -> y: (32, 16, 16384) f32.

Sharding: 2 channels per core (C=16 over 8 cores), full batch B=32 and full
T per core - pure data parallelism, no collectives.

Algorithm (v2, 21250ns modeled vs 33132ns v1 baseline):
  1. T axis cut into NSEG=8 chains per channel with a DELTA=128 zero-state
     warm-up block (truncation < 1e-7); 16 chains/core, 17 blocks of L=128.
  2. Per block the host packs D' = 16*(N - I) (N = in-block banded taps,
     diagonal = -16 folded in) plus the 32-col coupling block S as one dense
     fp8 [128 x 16ch x 160B] tile, streamed over a ring of 9 SBUF tiles.
     (A skew-packed 33-wide band DMA is 2.4x less traffic and works in
     CoreSim, but per-partition-offset access patterns lower incorrectly to
     real DMA descriptors - validated broken on hardware - so dense it is.)
  3. Two-step block solve, seeded WITHOUT the coupling term, so no serial
     inter-block dependency chain exists (the missing S reaches the chained
     tail only at O(N^3); numerically validated):
       ps0 = D'@y0  (PE)        z = copy(ps0) = -16*y1   (ACT, bf16, no
                                 act-table: scale folds into the final 1/256)
       ps1 = D'@z + S@z_prev + 16*I@z   (PE, 4 matmuls/chain-block total)
       yf  = x + ps1/256        (DVE stt, bf16 out)
  4. x streamed bf16, output stored bf16 (upcast on host); end-to-end rel
     error 4.4e-3 vs the exact recurrence (hardware-validated; gate 2e-2).

Schedule: ps1 lags ps0 by PIPE=3 blocks so the in-order PE queue never
waits on ACT; per-queue DMA assignment balances SP/Pool/ACT(+ATL) at
~13.5us each against PE 13.8us busy (z-copies: ACT except 3 on DVE; they
must stay off GPSIMD, whose ops cannot read PSUM on real hardware);
stores are quad-block with a pair+singles tail; startup load order is
tuned to PE's in-order band consumption (band3 on SP ahead of band4).
"""

import sys

sys.path.insert(0, "/opt/trn_rl_repo")

import numpy as np
import ml_dtypes

from concourse import bacc, mybir
from concourse.bass_utils import run_bass_kernel_spmd
from concourse.tile import TileContext

BF16 = ml_dtypes.bfloat16
F8 = ml_dtypes.float8_e4m3fn

B, C, T = 32, 16, 16384
P = 32
L = 128
NCORES = 8
CLOC = C // NCORES      # 2
NSEG = 8
NCHAIN = CLOC * NSEG    # 16
SEGT = T // NSEG        # 2048
DELTA = 128
NBLK = SEGT // L + 1    # 17
NKEEP = NBLK - 1        # 16
ROW = L + P             # 160
NBAND = 9               # band-tile ring (dense reload each block)
PIPE = 3                # ps1 lags ps0 by PIPE blocks (keeps PE stall-free)

_last_exec_ns = None


def build_graph():
    nc = bacc.Bacc(detect_race_conditions=False)

    bd = nc.declare_dram_parameter(
        "bd", [NBLK, L, NCHAIN, ROW], mybir.dt.float8e4, isOutput=False
    )
    xw = nc.declare_dram_parameter(
        "xw",
        [(NBLK + 1) // 2, L, 2, NCHAIN, B],
        mybir.dt.bfloat16,
        isOutput=False,
    )
    i16 = nc.declare_dram_parameter(
        "i16", [L, L], mybir.dt.bfloat16, isOutput=False
    )
    out = nc.declare_dram_parameter(
        "out",
        [NKEEP // 4, L, 4, NCHAIN, B],
        mybir.dt.bfloat16,
        isOutput=True,
    )

    with TileContext(nc) as tc:
        with (
            tc.tile_pool(name="cst", bufs=1) as cst,
            tc.tile_pool(name="sb", bufs=9) as sb,
            tc.tile_pool(name="lp", bufs=3) as lp,
            tc.tile_pool(name="ps", bufs=4, space="PSUM") as ps,
        ):
            i16t = cst.tile([L, L], mybir.dt.bfloat16, tag="i16")
            band = [
                cst.tile(
                    [L, NCHAIN, ROW],
                    mybir.dt.float8e4,
                    tag=f"bd{i}",
                    name=f"band{i}",
                )
                for i in range(NBAND)
            ]
            xwts = [None] * NBLK
            xwp = [None] * (NBLK // 2 + 1)

            def load_xwp(p_, split=False):
                t_ = lp.tile(
                    [L, 2, NCHAIN, B],
                    mybir.dt.bfloat16,
                    tag="xwt",
                    bufs=7,
                    name=f"xwp{p_}",
                )
                n_ = min(2 * p_ + 2, NBLK) - 2 * p_
                if split:
                    for h_ in range(n_):
                        nc.gpsimd.dma_start(
                            out=t_[:, h_ : h_ + 1], in_=xw[p_, :, h_ : h_ + 1]
                        )
                else:
                    xeng = {
                        1: nc.gpsimd,
                        2: nc.sync, 6: nc.sync, 8: nc.sync,
                        3: nc.scalar, 4: nc.gpsimd, 5: nc.gpsimd,
                        7: nc.gpsimd,
                    }[p_]
                    xeng.dma_start(out=t_[:, 0:n_], in_=xw[p_, :, 0:n_])
                xwp[p_] = t_
                xwts[2 * p_] = (t_, 0)
                if 2 * p_ + 1 < NBLK:
                    xwts[2 * p_ + 1] = (t_, 1)

            nc.gpsimd.dma_start(out=band[0][:, 8:16], in_=bd[0, :, 8:16])
            load_xwp(0, split=True)
            load_xwp(1)
            nc.scalar.dma_start(out=band[2][:], in_=bd[2])
            nc.scalar.dma_start(out=i16t[:], in_=i16[:])

            y1s = [None] * NBLK
            yfp = [None] * (NKEEP // 2)
            for s in range(NBLK + PIPE):
                if s < NBLK:
                    bt = band[s % NBAND]
                    if True:
                        # dense band+S load (zeros included; skewed APs do not
                        # lower correctly to hardware descriptors)
                        beng = {
                            9: nc.scalar, 11: nc.scalar,
                            1: nc.sync, 2: nc.sync, 3: nc.sync, 6: nc.sync,
                            8: nc.sync, 13: nc.sync, 15: nc.sync,
                            16: nc.sync,
                            4: nc.gpsimd, 5: nc.gpsimd, 7: nc.gpsimd,
                            10: nc.gpsimd, 12: nc.gpsimd, 14: nc.gpsimd,
                        }

                        if s == 0:
                            nc.sync.dma_start(
                                out=bt[:, 0:8], in_=bd[0, :, 0:8]
                            )
                        elif s != 2:
                            beng[s].dma_start(out=bt[:], in_=bd[s])
                    if s % 2 == 1 and (s + 3) // 2 <= (NBLK - 1) // 2:
                        load_xwp((s + 3) // 2)
                    xwt, xh = xwts[s]

                    # ps0 = D'@y0 ; y1 = -ps0/16 (ACT, bf16)
                    ps0 = ps.tile(
                        [L, NCHAIN, B], mybir.dt.float32, tag="ps0", bufs=5
                    )
                    for ch in range(NCHAIN):
                        nc.tensor.matmul(
                            ps0[:, ch, :],
                            bt[:, ch, 0:L],
                            xwt[:, xh, ch, :],
                            start=True,
                            stop=True,
                        )
                    # z = ps0 = -16*y1 (plain copy, no act table); the /256
                    # compensation folds into the final DVE scalar
                    y1 = sb.tile(
                        [L, NCHAIN, B], mybir.dt.bfloat16, tag="y1", bufs=6
                    )
                    if s in (1, 2, 3, 4):
                        nc.vector.tensor_copy(y1[:], ps0[:])
                    else:
                        nc.scalar.copy(y1[:], ps0[:])
                    y1s[s] = y1

                # ps1(t) = D'@y1 + S_prev@y1_prev + 16*I@y1, t = s - PIPE
                # (t=0 is the discarded warm-up block: no ps1/yf at all)
                t = s - PIPE
                if t < 1:
                    continue
                btt = band[t % NBAND]
                y1 = y1s[t]
                ps1 = ps.tile(
                    [L, NCHAIN, B], mybir.dt.float32, tag="ps1", bufs=3
                )
                for ch in range(NCHAIN):
                    nc.tensor.matmul(
                        ps1[:, ch, :],
                        btt[:, ch, 0:L],
                        y1[:, ch, :],
                        start=True,
                        stop=False,
                        skip_group_check=True,
                    )
                    nc.tensor.matmul(
                        ps1[:, ch, :],
                        i16t[:],
                        y1[:, ch, :],
                        start=False,
                        stop=False,
                        skip_group_check=True,
                    )
                    nc.tensor.matmul(
                        ps1[0:P, ch, :],
                        band[(t - 1) % NBAND][:, ch, L:ROW],
                        y1s[t - 1][:, ch, :],
                        start=False,
                        stop=True,
                        skip_group_check=True,
                    )

                # yf = x + ps1/256 (DVE, bf16 out); quad-block stores
                ki = t - 1
                pp, hh = ki // 4, ki % 4
                if hh == 0:
                    yfp[pp] = sb.tile(
                        [L, 4, NCHAIN, B],
                        mybir.dt.bfloat16,
                        tag="yf",
                        bufs=2,
                        name=f"yf{pp}",
                    )
                xt, xth = xwts[t]
                yf_eng = nc.vector
                yf_eng.scalar_tensor_tensor(
                    out=yfp[pp][:, hh],
                    in0=ps1[:],
                    scalar=0.00390625,
                    in1=xt[:, xth],
                    op0=mybir.AluOpType.mult,
                    op1=mybir.AluOpType.add,
                )
                if pp < 3 and hh == 3:
                    eng = {0: nc.gpsimd, 1: nc.sync, 2: nc.gpsimd}[pp]
                    eng.dma_start(out=out[pp], in_=yfp[pp][:])
                elif pp == 3 and hh == 1:
                    nc.sync.dma_start(
                        out=out[pp, :, 0:2], in_=yfp[pp][:, 0:2]
                    )
                elif pp == 3 and hh == 2:
                    nc.gpsimd.dma_start(
                        out=out[pp, :, 2:3], in_=yfp[pp][:, 2:3]
                    )
                elif pp == 3 and hh == 3:
                    nc.sync.dma_start(
                        out=out[pp, :, 3:4], in_=yfp[pp][:, 3:4]
                    )
    return nc


def _host_prep(x, a):
    x = np.asarray(x, np.float32)
    a = np.asarray(a, np.float32)
    a1 = a[:, 1:, :] / a[:, :1, :]           # (C, 32, T)
    PAD0 = DELTA
    TP = T + PAD0 + L + P
    az = np.zeros((C, P, TP), np.float32)
    az[:, :, PAD0 : PAD0 + T] = a1
    xpad = np.zeros((B, C, TP), np.float32)
    xpad[:, :, PAD0 : PAD0 + T] = x

    k = np.arange(L)
    j = np.arange(1, P + 1)
    seg = np.arange(NSEG)
    s_ = np.arange(NBLK)
    # chain ch = cl*NSEG + g ; t0[g, s] = g*SEGT - DELTA + s*L
    t0 = seg[:, None] * SEGT - DELTA + s_[None, :] * L    # (NSEG, NBLK)

    # band values: BV[c, g, s, k, j-1] = 16*a_j(t0+k+j) (fp8)
    tidx = (
        t0[None, :, :, None, None]
        + k[None, None, None, :, None]
        + j[None, None, None, None, :]
        + PAD0
    )                                                      # (1,NSEG,NBLK,L,P)
    cidx = np.arange(C)[:, None, None, None, None]
    BV = (az[cidx, (j - 1)[None, None, None, None, :], tidx] * 16.0).astype(F8)

    # xw: XW[c, g, s, k, b] bf16
    txi = t0[None, :, :, None] + k[None, None, None, :] + PAD0
    XWf = xpad[:, np.arange(C)[:, None, None, None], txi]  # (B,C,NSEG,NBLK,L)
    XWf = np.moveaxis(XWf, 0, -1).astype(BF16)             # (C,NSEG,NBLK,L,B)

    i16c = (16.0 * np.eye(L)).astype(BF16)

    # dense-scatter indices: band value j of partition k -> col k+j
    sidx = np.arange(NBLK)[:, None, None, None]
    kidx = np.arange(L)[None, :, None, None]
    chidx = np.arange(NCHAIN)[None, None, :, None]
    colidx = (np.arange(L)[:, None] + np.arange(P + 1)[None, :])[
        None, :, None, :
    ]

    in_maps = []
    for r in range(NCORES):
        # BD[s, k, ch, j]: ch = cl*NSEG + g, channel c = 2r + cl
        BD = np.empty((NBLK, L, NCHAIN, P + 1), F8)
        BD[..., 0] = np.float32(-16.0)
        XW = np.zeros((NBLK + 1, L, NCHAIN, B), BF16)
        for cl in range(CLOC):
            c = 2 * r + cl
            # BV[c] : (NSEG, NBLK, L, P) -> [s, k, g, j]
            BD[:, :, cl * NSEG : (cl + 1) * NSEG, 1:] = BV[c].transpose(
                1, 2, 0, 3
            )
            XW[:NBLK, :, cl * NSEG : (cl + 1) * NSEG, :] = XWf[c].transpose(
                1, 2, 0, 3
            )
        XWP = np.ascontiguousarray(
            XW.reshape((NBLK + 1) // 2, 2, L, NCHAIN, B).transpose(
                0, 2, 1, 3, 4
            )
        )
        DNS = np.zeros((NBLK, L, NCHAIN, ROW), F8)
        DNS[sidx, kidx, chidx, colidx] = BD
        in_maps.append({"bd": DNS, "xw": XWP, "i16": i16c})
    return in_maps


def _assemble(results):
    y = np.empty((B, C, T), np.float32)
    for r in range(NCORES):
        o = np.asarray(results[r]["out"], dtype=np.float32)
        # o[pp, k, hh, ch, b] -> [si=4pp+hh, k, ch, b]
        o = o.transpose(0, 2, 1, 3, 4).reshape(NKEEP, L, CLOC, NSEG, B)
        # -> (b, cl, g, si, k)
        o = o.transpose(4, 2, 3, 0, 1).reshape(B, CLOC, T)
        y[:, 2 * r : 2 * r + CLOC, :] = o
    return y


def kernel(x, a):
    global _last_exec_ns
    nc = build_graph()
    if not nc.is_finalized():
        nc.finalize()
    in_maps = _host_prep(x, a)
    res = run_bass_kernel_spmd(nc, in_maps, core_ids=list(range(NCORES)))
    _last_exec_ns = res.exec_time_ns
    return _assemble(res.results)
